# revision 1
# baseline (speedup 1.0000x reference)
"""Causal self-attention Trainium2 Bass kernel.

Problem: B=4, T=2048, DIM=1024, H=16 heads, head_dim=64 (fp32).
  qkv = x @ w_qkv.T ; per-head causal softmax(q k^T / 8) v ; out @ w_out.T

Sharding (8 cores): core c -> (batch b = c//2, head-group g = c%2 of 8 heads).
Each core computes a partial output y_partial = attn_out_g @ w_out[:, g]^T
for its batch; host sums the two head-group partials per batch.

Device layout (per core):
  xt      [1024, 2048] f32r : x[b]^T (dim-major)          -- host-transposed
  wqkvt   [1024, 1536] f32r : [Wq|Wk|Wv]^T slice          -- host-transposed
  woutt   [ 512, 1024] f32r : w_out[:, g]^T               -- host-transposed
  masks   [ 128, 2048] bf16 : 4 causal kill-triangles (1.0 = masked out)
  negdiag [ 128,  128] bf16 : diag(-1e30) -- routes kill-triangles into PSUM
  yt      [1024, 2048] f32  : partial output, transposed

Pipeline per token-chunk c (512 tokens), fully interleaved so PE keeps busy
while ScalarE runs the exp stream:
  1. QKV projection -> QT/KT (head-dim major, f32r) and V (token major, bf16,
     with a ones column per head that makes P@V also emit the softmax
     denominator row).
  2. Attention for q-chunk c: transposed scores for 2 heads x 2 ktiles per
     PSUM quad (row-packed via base_partition 0/64 so the K=64 matmuls run
     concurrently); causal masking is an extra matmul accumulating -1e30
     kill-triangles into the quad before exp; one [128,2048] exp on ScalarE
     (scale=1/8 folded in, no max-subtraction; |scores| small so fp32 exp is
     safe); P@V accumulates per-head output plus denominator row; divide via
     fast-reciprocal + PE broadcast + vector multiply.
  3. Output projection of the finished 512-token chunk.
"""

import contextlib

import numpy as np
import ml_dtypes

import concourse.bass as bass
import concourse.mybir as mybir
import concourse.tile as tile
from concourse import bacc
from concourse.bass_utils import run_bass_kernel_spmd

B, T, DIM = 4, 2048, 1024
NUM_HEADS, HEAD_DIM = 16, 64
INNER = NUM_HEADS * HEAD_DIM
SCALE = HEAD_DIM ** -0.5

N_CORES = 8
HEADS_PER_CORE = 8
HG = HEADS_PER_CORE * HEAD_DIM  # 512 = inner slice per core
NCH = T // 512                  # 4 token chunks
KT_PER_CH = 4                   # 128-ktok tiles per 512 chunk

F32R = mybir.dt.float32r
F32 = mybir.dt.float32
BF16 = mybir.dt.bfloat16


def build_bass():
    nc = bacc.Bacc()
    xt = nc.declare_dram_parameter("xt", [DIM, T], BF16, isOutput=False)
    wqkvt = nc.declare_dram_parameter("wqkvt", [DIM, 3 * HG], BF16, isOutput=False)
    woutt = nc.declare_dram_parameter("woutt", [HG, DIM], BF16, isOutput=False)
    masks = nc.declare_dram_parameter("masks", [128, 4096], BF16, isOutput=False)
    vones = nc.declare_dram_parameter("vones", [128, 8 * 65], BF16, isOutput=False)
    yt = nc.declare_dram_parameter("yt", [DIM, T], F32, isOutput=True)

    with tile.TileContext(nc) as tc:
        _emit(nc, tc, xt, wqkvt, woutt, masks, vones, yt)
    nc.finalize()
    return nc


def _emit(nc, tc, xt, wqkvt, woutt, masks, vones, yt):
    ctx = contextlib.ExitStack()
    with ctx:
        singles = ctx.enter_context(tc.tile_pool(name="singles", bufs=1))
        xpool = ctx.enter_context(tc.tile_pool(name="xpool", bufs=16))
        epool = ctx.enter_context(tc.tile_pool(name="epool", bufs=3))
        apool = ctx.enter_context(tc.tile_pool(name="apool", bufs=1))
        spool = ctx.enter_context(tc.tile_pool(name="spool", bufs=1))
        dpool = ctx.enter_context(tc.tile_pool(name="dpool", bufs=2, space="DRAM"))
        # PSUM budget (8 banks of 2KB/partition):
        #   pair [128,1024] bufs=2 -> 4 banks (scores, double-buffered)
        #   ot   [65,512]  3 slots -> 3 banks (otA/otB rotate)
        #   qkv  [128,512] bufs=1  -> 1 bank (stage 1 + stage 3 groups)
        psq = ctx.enter_context(tc.tile_pool(name="psq", bufs=2, space="PSUM"))
        psot = ctx.enter_context(tc.tile_pool(name="psot", bufs=3, space="PSUM"))
        psmm = ctx.enter_context(tc.tile_pool(name="psmm", bufs=1, space="PSUM"))

        # ---- persistent SBUF tensors ----
        wq = []
        for k in range(8):
            w = singles.tile([128, 3 * HG], BF16, name=f"wq{k}")
            nc.sync.dma_start(out=w, in_=wqkvt[k * 128:(k + 1) * 128, :])
            wq.append(w)
        wo = []
        for k in range(4):
            w = singles.tile([128, DIM], BF16, name=f"wo{k}")
            nc.sync.dma_start(out=w, in_=woutt[k * 128:(k + 1) * 128, :])
            wo.append(w)
        msk = singles.tile([128, 4096], BF16, name="msk")
        nc.sync.dma_start(out=msk, in_=masks[:, :])

        # QT/KT: 4 tiles [128, 2048] (2 heads per tile, head-dim major)
        qt = [singles.tile([128, T], BF16, name=f"qt{m}") for m in range(4)]
        kt = [singles.tile([128, T], BF16, name=f"kt{m}") for m in range(4)]
        # V: 16 token-tiles [128, 8*65] bf16 (per head: 64 v-cols + ones col)
        vt = [singles.tile([128, HEADS_PER_CORE * 65], BF16, name=f"vt{t}")
              for t in range(16)]
        for t in range(16):
            nc.sync.dma_start(out=vt[t], in_=vones[:, :])

        def stage1(c):
            cs = slice(c * 512, (c + 1) * 512)
            xts = []
            for k in range(8):
                xtile = xpool.tile([128, 512], BF16, tag="xt", name=f"x{c}_{k}")
                nc.sync.dma_start(out=xtile, in_=xt[k * 128:(k + 1) * 128, cs])
                xts.append(xtile)
            for which, dst in ((0, qt), (1, kt)):
                for m in range(4):
                    ps = psmm.tile([128, 512], F32, tag="qkv", name=f"pq{c}{which}{m}")
                    for k in range(8):
                        nc.tensor.matmul(
                            ps,
                            lhsT=wq[k][:, which * HG + m * 128: which * HG + (m + 1) * 128],
                            rhs=xts[k],
                            start=(k == 0), stop=(k == 7),
                        )
                    nc.vector.tensor_copy(dst[m][:, cs], ps)
            for i in range(4):
                t = c * 4 + i
                ps = psmm.tile([128, 512], F32, tag="qkv", name=f"pv{t}")
                for k in range(8):
                    nc.tensor.matmul(
                        ps,
                        lhsT=xts[k][:, i * 128:(i + 1) * 128],
                        rhs=wq[k][:, 2 * HG:3 * HG],
                        start=(k == 0), stop=(k == 7),
                    )
                v3 = vt[t].rearrange("p (h d) -> p h d", h=HEADS_PER_CORE)
                nc.vector.tensor_copy(
                    v3[:, :, 0:64],
                    ps.rearrange("p (h d) -> p h d", h=HEADS_PER_CORE))

        def attention(c):
            cs = slice(c * 512, (c + 1) * 512)
            n_kt = KT_PER_CH * (c + 1)
            aot = [apool.tile([128, 512], BF16, tag=f"aot{k}", name=f"aot{c}_{k}")
                   for k in range(4)]
            for hp in range(4):            # head pair (2hp, 2hp+1)
                hA, hB = 2 * hp, 2 * hp + 1
                otA = psot.tile([65, 512], F32, tag="ot", name=f"otA{c}_{hp}")
                otB = psot.tile([65, 512], F32, tag="ot", name=f"otB{c}_{hp}")
                for tk in range(n_kt):  # pair: ktile tk x 2 heads
                    q = psq.tile([128, 1024], F32, tag="pair", name=f"s{c}_{hp}_{tk}")
                    diag = tk >= n_kt - 4
                    for i in range(2):
                        ho = i * 64
                        nc.tensor.matmul(
                            q[:, i * 512:(i + 1) * 512],
                            lhsT=kt[hp][ho:ho + 64, tk * 128:(tk + 1) * 128],
                            rhs=qt[hp][ho:ho + 64, cs],
                            start=True, stop=True,
                            tile_position=(ho, 0),
                        )
                    e = epool.tile([128, 1024], BF16, tag="e", name=f"e{c}_{hp}_{tk}")
                    nc.scalar.activation(e, q, mybir.ActivationFunctionType.Exp,
                                         scale=float(SCALE))
                    if diag:
                        j = tk - (n_kt - 4)
                        nc.vector.tensor_mul(e, e, msk[:, j * 1024:(j + 1) * 1024])
                    for i, h in ((0, hA), (1, hB)):
                        nc.tensor.matmul(
                            otA if i == 0 else otB,
                            lhsT=vt[tk][:, h * 65:h * 65 + 65],
                            rhs=e[:, i * 512:(i + 1) * 512],
                            start=(tk == 0), stop=(tk == n_kt - 1),
                        )
                for (h, ot) in ((hA, otA), (hB, otB)):
                    den = spool.tile([1, 512], F32, tag="den", name=f"dn{c}_{h}")
                    nc.vector.tensor_copy(den, ot[64:65, :])
                    recf = spool.tile([1, 512], F32, tag="recf", name=f"rf{c}_{h}")
                    nc.vector.reciprocal_approx_fast(recf, den)
                    dr = dpool.tile([1, 512], F32, tag="dr", name=f"dr{c}_{h}")
                    nc.sync.dma_start(out=dr, in_=recf)
                    bcs = spool.tile([64, 512], F32, tag="bcs", bufs=2,
                                     name=f"bs{c}_{h}")
                    nc.sync.dma_start(out=bcs, in_=dr.to_broadcast((64, 512)))
                    nc.vector.tensor_mul(
                        aot[hp][(h % 2) * 64:(h % 2) * 64 + 64, :],
                        ot[0:64, :], bcs)
            return aot

        def stage3(c, aot):
            cs = slice(c * 512, (c + 1) * 512)
            for od in range(8):
                ps = psmm.tile([128, 512], F32, tag="qkv", name=f"py{c}_{od}")
                for k in range(4):
                    nc.tensor.matmul(
                        ps,
                        lhsT=wo[k][:, od * 128:(od + 1) * 128],
                        rhs=aot[k],
                        start=(k == 0), stop=(k == 3),
                    )
                ys = spool.tile([128, 512], F32, tag="ys", bufs=2, name=f"ys{c}_{od}")
                nc.vector.tensor_copy(ys, ps)
                nc.sync.dma_start(out=yt[od * 128:(od + 1) * 128, cs], in_=ys)

        # stage1(c+1) is emitted between attention(c) and stage3(c): the PE
        # queue then has QKV matmuls to chew on while the last head-pair's
        # division chain (DVE + DMA broadcast) finishes, instead of stalling
        # in-order on stage3's first accumulation group.
        stage1(0)
        for c in range(NCH):
            aot = attention(c)
            if c + 1 < NCH:
                stage1(c + 1)
            stage3(c, aot)


_NC_CACHE = None


def _get_nc():
    global _NC_CACHE
    if _NC_CACHE is None:
        _NC_CACHE = build_bass()
    return _NC_CACHE


def make_masks():
    """Keep masks: block j is [m_j | m_j] with m_j[k, q] = 1.0 iff q >= k + 128j."""
    k = np.arange(128)[:, None]
    q = np.arange(512)[None, :]
    m = np.zeros((128, 4096), dtype=np.float32)
    for j in range(4):
        keep = (q >= k + 128 * j)
        m[:, j * 1024:j * 1024 + 512] = keep
        m[:, j * 1024 + 512:(j + 1) * 1024] = keep
    return m.astype(ml_dtypes.bfloat16)


def make_in_maps(x, w_qkv, w_out):
    x = np.asarray(x, dtype=np.float32)
    w_qkv = np.asarray(w_qkv, dtype=np.float32)
    w_out = np.asarray(w_out, dtype=np.float32)
    msk = make_masks()
    in_maps = []
    for c in range(N_CORES):
        b, g = c // 2, c % 2
        gs = slice(g * HG, (g + 1) * HG)
        wsel = np.concatenate(
            [w_qkv[0 * INNER:][gs], w_qkv[1 * INNER:][gs], w_qkv[2 * INNER:][gs]],
            axis=0)                               # [1536, 1024]
        in_maps.append({
            "xt": np.ascontiguousarray(x[b].T).astype(ml_dtypes.bfloat16),
            "wqkvt": np.ascontiguousarray(wsel.T).astype(ml_dtypes.bfloat16),
            "woutt": np.ascontiguousarray(w_out[:, gs].T).astype(ml_dtypes.bfloat16),
            "masks": msk,
            "vones": np.ones((128, 8 * 65), dtype=ml_dtypes.bfloat16),
        })
    return in_maps


def kernel(x, mask, w_qkv, w_out, **_):
    nc = _get_nc()
    in_maps = make_in_maps(x, w_qkv, w_out)
    res = run_bass_kernel_spmd(nc, in_maps, core_ids=list(range(N_CORES)))
    y = np.zeros((B, T, DIM), dtype=np.float32)
    for c in range(N_CORES):
        y[c // 2] += res.results[c]["yt"].T
    return y



# revision 10
# speedup vs baseline: 1.0612x; 1.0612x over previous
"""Causal self-attention Trainium2 Bass kernel (fp8 DoubleRow version).

Problem: B=4, T=2048, DIM=1024, H=16 heads, head_dim=64 (fp32).
  qkv = x @ w_qkv.T ; per-head causal softmax(q k^T / 8) v ; out @ w_out.T

Sharding (8 cores): core c -> (batch b = c//2, head-group g = c%2 of 8 heads).
Each core computes a partial output y_partial = attn_out_g @ w_out[:, g]^T
for its batch; host sums the two head-group partials per batch.

Precision scheme (validated vs fp32 reference, rel max err ~1e-2 < 2e-2):
  - QKV projection in fp8e4 (e4m3) with MatmulPerfMode.DoubleRow: each
    matmul contracts 2 k-subtiles of 128 at 0.5 cycles/col.
  - Scores in fp8 DoubleRow: q/k stored [32 part, 2 dim-half, tokens]
    per 4-head group (weight columns pre-ordered on host), K = 32 x 2.
  - PV in fp8 DoubleRow over key-tile pairs: lhsT = v [128, 2, 65]
    (65th column of ones emits the softmax denominator row).
  - exp computes exp(s/8 - 2): the -2 bias keeps e < 240 (fp8 max);
    it cancels in the normalization.
  - Chunk-0 queries (few attended keys -> fp8 noise doesn't average
    out) use a full bf16 path: bf16 QKV for chunk 0 + bf16 scores/PV.
  - Output projection in bf16.

Causal masking: a -240 "kill rectangle/triangle" is accumulated into the
scores PSUM by an extra matmul (lhsT = diag(-240), rhs = 0/1 pattern);
exp then underflows to exactly 0 (fp8) / ~1e-12 (bf16).  Diagonal
chunks are column-sliced to the live query union of each ktile pair,
so scores/exp/PV skip most fully-masked work.
"""

import contextlib
from collections import deque

import numpy as np
import ml_dtypes

import concourse.bass as bass
import concourse.mybir as mybir
import concourse.tile as tile
from concourse import bacc
from concourse.bass_utils import run_bass_kernel_spmd

B, T, DIM = 4, 2048, 1024
NUM_HEADS, HEAD_DIM = 16, 64
INNER = NUM_HEADS * HEAD_DIM
SCALE = HEAD_DIM ** -0.5

N_CORES = 8
HEADS_PER_CORE = 8
HG = HEADS_PER_CORE * HEAD_DIM  # 512 = inner slice per core
NCH = T // 512                  # 4 token chunks

F32 = mybir.dt.float32
BF16 = mybir.dt.bfloat16
F8 = mybir.dt.float8e4
DR = mybir.MatmulPerfMode.DoubleRow

NEG = -240.0
EXP_BIAS = -2.0


def xr(ap, pattern, **kw):
    return ap.rearrange(pattern, **kw)


def build_bass():
    nc = bacc.Bacc()
    xtbf = nc.declare_dram_parameter("xtbf", [DIM, 512], BF16, isOutput=False)
    xt8 = nc.declare_dram_parameter("xt8", [512, 2 * T], F8, isOutput=False)
    wqbf = nc.declare_dram_parameter("wqbf", [DIM, 3 * HG], BF16, isOutput=False)
    wq8 = nc.declare_dram_parameter("wq8", [512, 2 * 3 * HG], F8, isOutput=False)
    woutt = nc.declare_dram_parameter("woutt", [HG, DIM], BF16, isOutput=False)
    trif8 = nc.declare_dram_parameter("trif8", [128, 384], F8, isOutput=False)
    tribf = nc.declare_dram_parameter("tribf", [128, 384], BF16, isOutput=False)
    negd8 = nc.declare_dram_parameter("negd8", [128, 128], F8, isOutput=False)
    negdbf = nc.declare_dram_parameter("negdbf", [128, 128], BF16, isOutput=False)
    yt = nc.declare_dram_parameter("yt", [DIM, T], F32, isOutput=True)

    with tile.TileContext(nc) as tc:
        _emit(nc, tc, xtbf, xt8, wqbf, wq8, woutt, trif8, tribf, negd8,
              negdbf, yt)
    nc.finalize()
    return nc


def _emit(nc, tc, xtbf, xt8, wqbf, wq8, woutt, trif8, tribf, negd8, negdbf,
          yt):
    ctx = contextlib.ExitStack()
    with ctx:
        singles = ctx.enter_context(tc.tile_pool(name="singles", bufs=1))
        xpool = ctx.enter_context(tc.tile_pool(name="xpool", bufs=2))
        epool = ctx.enter_context(tc.tile_pool(name="epool", bufs=3))
        apool = ctx.enter_context(tc.tile_pool(name="apool", bufs=2))
        spool = ctx.enter_context(tc.tile_pool(name="spool", bufs=1))
        dpool = ctx.enter_context(tc.tile_pool(name="dpool", bufs=2, space="DRAM"))
        # PSUM budget (8 banks of 2KB/partition):
        #   psq  [128,1024] bufs=2 -> 4 banks (score quads, double-buffered)
        #   psot [65,512]   bufs=3 -> 3 banks (per-head PV accumulators)
        #   psmm [128,512]  bufs=1 -> 1 bank (stage 1 + stage 3 groups)
        psq = ctx.enter_context(tc.tile_pool(name="psq", bufs=2, space="PSUM"))
        psot = ctx.enter_context(tc.tile_pool(name="psot", bufs=3, space="PSUM"))
        psmm = ctx.enter_context(tc.tile_pool(name="psmm", bufs=1, space="PSUM"))

        # ---- persistent SBUF tensors; DMA order = dependency order ----
        wqb = []
        for k in range(8):
            w = singles.tile([128, 3 * HG], BF16, name=f"wqb{k}")
            nc.sync.dma_start(out=w, in_=wqbf[k * 128:(k + 1) * 128, :])
            wqb.append(w)
        # chunk-0 x, needed immediately after the bf16 weights
        xts0 = []
        for k in range(8):
            xtile = xpool.tile([128, 512], BF16, tag=f"xb{k}", name=f"xb{k}")
            nc.sync.dma_start(out=xtile, in_=xtbf[k * 128:(k + 1) * 128, :])
            xts0.append(xtile)
        wq8s = []
        for k in range(4):
            w = singles.tile([128, 2, 3 * HG], F8, name=f"wq8{k}")
            nc.sync.dma_start(out=xr(w, "p s c -> p (s c)"),
                              in_=wq8[k * 128:(k + 1) * 128, :])
            wq8s.append(w)

        def x8_tiles(c):
            cs = slice(c * 512, (c + 1) * 512)
            x8s = []
            for k in range(4):
                t8 = xpool.tile([128, 2, 512], F8, tag=f"x8_{k}",
                                name=f"x8_{c}_{k}")
                nc.sync.dma_start(
                    out=t8,
                    in_=xr(xt8[k * 128:(k + 1) * 128, :],
                           "p (s t) -> p s t", s=2)[:, :, cs])
                x8s.append(t8)
            return x8s

        x8s0 = x8_tiles(0)

        wo = []
        for k in range(4):
            w = singles.tile([128, DIM], BF16, name=f"wo{k}")
            nc.sync.dma_start(out=w, in_=woutt[k * 128:(k + 1) * 128, :])
            wo.append(w)
        tri8 = singles.tile([128, 384], F8, name="tri8")
        nc.sync.dma_start(out=tri8, in_=trif8[:, :])
        trib = singles.tile([128, 384], BF16, name="trib")
        nc.sync.dma_start(out=trib, in_=tribf[:, :])
        nd8 = singles.tile([128, 128], F8, name="nd8")
        nc.sync.dma_start(out=nd8, in_=negd8[:, :])
        ndb = singles.tile([128, 128], BF16, name="ndb")
        nc.sync.dma_start(out=ndb, in_=negdbf[:, :])
        biasap = singles.tile([128, 1], F32, name="expbias")
        nc.gpsimd.memset(biasap, EXP_BIAS)

        # chunk-0 bf16 q/k: 4 tiles [128, 512] (2 heads per tile)
        qtb = [singles.tile([128, 512], BF16, name=f"qtb{m}") for m in range(4)]
        ktb = [singles.tile([128, 512], BF16, name=f"ktb{m}") for m in range(4)]
        # chunk-0 bf16 v: 4 token-tiles [128, 8, 65]
        vtb = [singles.tile([128, HEADS_PER_CORE, 65], BF16, name=f"vtb{t}")
               for t in range(4)]
        for t in range(4):
            nc.gpsimd.memset(vtb[t][:, :, 64:65], 1.0)
        # fp8 q (chunks 1-3): 2 head-group tiles [4h x 32d, 2 dim-half, 1536]
        qt8 = [singles.tile([128, 2, 3 * 512], F8, name=f"qt8{g}")
               for g in range(2)]
        # fp8 k (all chunks): [128, 2, 2048]
        kt8 = [singles.tile([128, 2, T], F8, name=f"kt8{g}") for g in range(2)]
        # fp8 v: 8 ktile-pair tiles [128 tok, 8 heads, 2 sub-ktile, 65]
        vt8 = [singles.tile([128, HEADS_PER_CORE, 2, 128], F8, name=f"vt8{tp}")
               for tp in range(8)]
        for tp in range(8):
            nc.gpsimd.memset(vt8[tp][:, :, :, 64:65], 1.0)
            nc.gpsimd.memset(vt8[tp][:, :, :, 65:128], 0.0)

        def stage1_ch0():
            """bf16 QKV for chunk 0 + fp8 k/v for chunk 0."""
            for which, dst in ((0, qtb), (1, ktb)):
                for m in range(4):
                    ps = psmm.tile([128, 512], F32, tag="qkv", name=f"pb{which}{m}")
                    for k in range(8):
                        nc.tensor.matmul(
                            ps,
                            lhsT=wqb[k][:, which * HG + m * 128:
                                        which * HG + (m + 1) * 128],
                            rhs=xts0[k],
                            start=(k == 0), stop=(k == 7))
                    nc.vector.tensor_copy(dst[m], ps)
            for i in range(4):
                ps = psmm.tile([128, 512], F32, tag="qkv", name=f"pbv{i}")
                for k in range(8):
                    nc.tensor.matmul(
                        ps,
                        lhsT=xts0[k][:, i * 128:(i + 1) * 128],
                        rhs=wqb[k][:, 2 * HG:3 * HG],
                        start=(k == 0), stop=(k == 7))
                nc.vector.tensor_copy(
                    vtb[i][:, :, 0:64],
                    xr(ps, "p (h d) -> p h d", h=HEADS_PER_CORE))
            for m in range(4):
                ps = psmm.tile([128, 512], F32, tag="qkv", name=f"p8k{m}")
                for k in range(4):
                    nc.tensor.matmul(
                        ps,
                        lhsT=wq8s[k][:, :, HG + m * 128:HG + (m + 1) * 128],
                        rhs=x8s0[k],
                        start=(k == 0), stop=(k == 3), perf_mode=DR)
                nc.vector.tensor_copy(kt8[m // 2][:, m % 2, 0:512], ps)
            for i in range(4):
                ps = psmm.tile([128, 512], F32, tag="qkv", name=f"p8v{i}")
                for k in range(4):
                    nc.tensor.matmul(
                        ps,
                        lhsT=x8s0[k][:, :, i * 128:(i + 1) * 128],
                        rhs=wq8s[k][:, :, 2 * HG:3 * HG],
                        start=(k == 0), stop=(k == 3), perf_mode=DR)
                nc.vector.tensor_copy(
                    vt8[i // 2][:, :, i % 2, 0:64],
                    xr(ps, "p (h d) -> p h d", h=HEADS_PER_CORE))

        def stage1_fp8_fillers(c):
            """fp8 QKV for chunk c>=1, as filler closures (4 matmuls each)."""
            cs = slice(c * 512, (c + 1) * 512)
            qcs = slice((c - 1) * 512, c * 512)
            x8s = x8_tiles(c)
            fillers = []

            def qk_group(which, m):
                def go():
                    ps = psmm.tile([128, 512], F32, tag="qkv",
                                   name=f"p8{which}_{c}_{m}")
                    for k in range(4):
                        nc.tensor.matmul(
                            ps,
                            lhsT=wq8s[k][:, :, which * HG + m * 128:
                                         which * HG + (m + 1) * 128],
                            rhs=x8s[k],
                            start=(k == 0), stop=(k == 3), perf_mode=DR)
                    dst = (qt8 if which == 0 else kt8)[m // 2]
                    nc.vector.tensor_copy(
                        dst[:, m % 2, qcs if which == 0 else cs], ps)
                return go

            def v_group(i):
                def go():
                    t = 4 * c + i
                    ps = psmm.tile([128, 512], F32, tag="qkv", name=f"p8V{c}{i}")
                    for k in range(4):
                        nc.tensor.matmul(
                            ps,
                            lhsT=x8s[k][:, :, i * 128:(i + 1) * 128],
                            rhs=wq8s[k][:, :, 2 * HG:3 * HG],
                            start=(k == 0), stop=(k == 3), perf_mode=DR)
                    nc.vector.tensor_copy(
                        vt8[t // 2][:, :, t % 2, 0:64],
                        xr(ps, "p (h d) -> p h d", h=HEADS_PER_CORE))
                return go

            for m in range(4):
                fillers.append(qk_group(1, m))
            for i in range(4):
                fillers.append(v_group(i))
            for m in range(4):
                fillers.append(qk_group(0, m))
            return fillers

        def _scores_bf(h, quad, tp, U, u0, diag):
            m, ho = h // 2, 64 * (h % 2)
            for sub in range(2):
                t = 2 * tp + sub
                lhsT = ktb[m][ho:ho + 64, t * 128:(t + 1) * 128]
                base = sub * U
                if not diag:
                    nc.tensor.matmul(
                        quad[:, base:base + U],
                        lhsT=lhsT, rhs=qtb[m][ho:ho + 64, u0:u0 + U],
                        start=True, stop=True, tile_position=(ho, 0))
                    continue
                lead = min(128 + sub * 128, U)
                nc.tensor.matmul(
                    quad[:, base:base + lead],
                    lhsT=lhsT, rhs=qtb[m][ho:ho + 64, u0:u0 + lead],
                    start=True, stop=False, tile_position=(ho, 0))
                tri_lo = 0 if sub == 0 else 128
                nc.tensor.matmul(
                    quad[:, base:base + lead],
                    lhsT=ndb, rhs=trib[:, tri_lo:tri_lo + lead],
                    start=False, stop=True)
                if lead < U:
                    nc.tensor.matmul(
                        quad[:, base + lead:base + U],
                        lhsT=lhsT, rhs=qtb[m][ho:ho + 64, u0 + lead:u0 + U],
                        start=True, stop=True, tile_position=(ho, 0))

        def _scores_f8(h, quad, tp, U, u0, c, diag):
            g, hi = h // 4, 32 * (h % 4)
            qg = qt8[g]
            qoff = (c - 1) * 512 + u0
            for sub in range(2):
                t = 2 * tp + sub
                lhsT = kt8[g][hi:hi + 32, :, t * 128:(t + 1) * 128]
                base = sub * U
                if not diag:
                    nc.tensor.matmul(
                        quad[:, base:base + U],
                        lhsT=lhsT, rhs=qg[hi:hi + 32, :, qoff:qoff + U],
                        start=True, stop=True, tile_position=(hi, 0),
                        perf_mode=DR)
                    continue
                lead = min(128 + sub * 128, U)
                nc.tensor.matmul(
                    quad[:, base:base + lead],
                    lhsT=lhsT, rhs=qg[hi:hi + 32, :, qoff:qoff + lead],
                    start=True, stop=False, tile_position=(hi, 0), perf_mode=DR)
                tri_lo = 0 if sub == 0 else 128
                nc.tensor.matmul(
                    quad[:, base:base + lead],
                    lhsT=nd8, rhs=tri8[:, tri_lo:tri_lo + lead],
                    start=False, stop=True)
                if lead < U:
                    nc.tensor.matmul(
                        quad[:, base + lead:base + U],
                        lhsT=lhsT, rhs=qg[hi:hi + 32, :, qoff + lead:qoff + U],
                        start=True, stop=True, tile_position=(hi, 0),
                        perf_mode=DR)

        def attention(c, fillers):
            aot = [apool.tile([128, 512], BF16, tag=f"aot{k}", name=f"aot{c}_{k}")
                   for k in range(4)]
            npairs = 2 * (c + 1)
            for h in range(8):
                ot = psot.tile([128, 512], F32, tag="ot", name=f"ot{c}_{h}")
                for tp in range(npairs):
                    diag = tp >= npairs - 2
                    U = 256 if (diag and tp == npairs - 1) else 512
                    u0 = 512 - U
                    quad = psq.tile([128, 1024], F32, tag="quad",
                                    name=f"q{c}_{h}_{tp}")
                    if c == 0:
                        _scores_bf(h, quad, tp, U, u0, diag)
                    else:
                        _scores_f8(h, quad, tp, U, u0, c, diag)
                    et = epool.tile([128, 1024], BF16 if c == 0 else F8,
                                    tag="eb" if c == 0 else "e8",
                                    name=f"e{c}_{h}_{tp}")
                    nc.scalar.activation(
                        et[:, 0:2 * U], quad[:, 0:2 * U],
                        mybir.ActivationFunctionType.Exp,
                        scale=float(SCALE), bias=biasap)
                    if c == 0:
                        for sub in range(2):
                            t = 2 * tp + sub
                            nc.tensor.matmul(
                                ot[0:65, u0:u0 + U],
                                lhsT=vtb[t][:, h, :],
                                rhs=et[:, sub * U:(sub + 1) * U],
                                start=(tp == 0 and sub == 0),
                                stop=(tp == npairs - 1 and sub == 1))
                    else:
                        nc.tensor.matmul(
                            ot[:, u0:u0 + U],
                            lhsT=vt8[tp][:, h, :, :],
                            rhs=xr(et[:, 0:2 * U], "p (s u) -> p s u", s=2),
                            start=(tp == 0), stop=(tp == npairs - 1),
                            perf_mode=DR)
                # normalize: row 64 of ot is the denominator
                den = spool.tile([1, 512], F32, tag="den", name=f"dn{c}_{h}")
                nc.vector.tensor_copy(den, ot[64:65, :])
                recf = spool.tile([1, 512], F32, tag="recf", name=f"rf{c}_{h}")
                nc.vector.reciprocal_approx_fast(recf, den)
                dr = dpool.tile([1, 512], F32, tag="dr", name=f"dr{c}_{h}")
                nc.sync.dma_start(out=dr, in_=recf)
                bcs = spool.tile([64, 512], F32, tag="bcs", bufs=2,
                                 name=f"bs{c}_{h}")
                nc.sync.dma_start(out=bcs, in_=dr.to_broadcast((64, 512)))
                nc.vector.tensor_mul(
                    aot[h // 2][(h % 2) * 64:(h % 2) * 64 + 64, :],
                    ot[0:64, :], bcs)
                # pump fillers so the PE queue has QKV/out-proj work while
                # the exp + normalize chains of this head drain
                for _ in range(3):
                    if fillers:
                        fillers.popleft()()
            while fillers:
                fillers.popleft()()
            return aot

        def stage3_fillers(c, aot):
            cs = slice(c * 512, (c + 1) * 512)
            fillers = []

            def out_group(od):
                def go():
                    ps = psmm.tile([128, 512], F32, tag="qkv", name=f"py{c}_{od}")
                    for k in range(4):
                        nc.tensor.matmul(
                            ps,
                            lhsT=wo[k][:, od * 128:(od + 1) * 128],
                            rhs=aot[k],
                            start=(k == 0), stop=(k == 3))
                    ys = spool.tile([128, 512], F32, tag="ys", bufs=2,
                                    name=f"ys{c}_{od}")
                    nc.vector.tensor_copy(ys, ps)
                    nc.sync.dma_start(out=yt[od * 128:(od + 1) * 128, cs], in_=ys)
                return go

            for od in range(8):
                fillers.append(out_group(od))
            return fillers

        # ---- schedule ----
        stage1_ch0()
        fillers = deque()
        for c in range(NCH):
            if c + 1 < NCH:
                fillers.extend(stage1_fp8_fillers(c + 1))
            aot = attention(c, fillers)
            fillers = deque(stage3_fillers(c, aot))
        while fillers:
            fillers.popleft()()


_NC_CACHE = None


def _get_nc():
    global _NC_CACHE
    if _NC_CACHE is None:
        _NC_CACHE = build_bass()
    return _NC_CACHE


def _tri_pattern():
    k = np.arange(128)[:, None]
    q = np.arange(128)[None, :]
    tri = (q < k).astype(np.float32)          # masked (kill) positions
    return np.concatenate([tri, np.ones((128, 128), np.float32), tri], axis=1)


def make_in_maps(x, w_qkv, w_out):
    x = np.asarray(x, dtype=np.float32)
    w_qkv = np.asarray(w_qkv, dtype=np.float32)
    w_out = np.asarray(w_out, dtype=np.float32)
    pat = _tri_pattern()
    negd = np.diag(np.full(128, NEG, np.float32))
    f8 = ml_dtypes.float8_e4m3
    bf = ml_dtypes.bfloat16

    in_maps = []
    for core in range(N_CORES):
        b, g = core // 2, core % 2
        gs = slice(g * HG, (g + 1) * HG)
        wsel = np.concatenate(
            [w_qkv[0 * INNER:][gs], w_qkv[1 * INNER:][gs], w_qkv[2 * INNER:][gs]],
            axis=0)                               # [1536, 1024] bf16 order
        # fp8 weight column order: q/k in (grp, dim-half) blocks of 4h x 32d
        cols = np.empty(3 * HG, np.int64)
        j = np.arange(HG)
        m, r = j // 128, j % 128
        hh = g * 8 + (m // 2) * 4 + r // 32
        d = (m % 2) * 32 + (r % 32)
        cols[0:HG] = hh * 64 + d
        cols[HG:2 * HG] = INNER + hh * 64 + d
        cols[2 * HG:] = 2 * INNER + (g * 8 + j // 64) * 64 + (j % 64)
        wsel8 = w_qkv[cols, :]                    # [1536, 1024]
        # wq8 dram [512, 3072]: row 128k+p, col i*1536+j = wsel8[j, 256k+128i+p]
        wq8d = wsel8.T.reshape(4, 2, 128, 3 * HG).transpose(0, 2, 1, 3)
        wq8d = np.ascontiguousarray(wq8d.reshape(512, 2 * 3 * HG))
        # xt8 dram [512, 4096]: row 128k+p, col i*2048+t = x[b][t, 256k+128i+p]
        xt8d = x[b].T.reshape(4, 2, 128, T).transpose(0, 2, 1, 3)
        xt8d = np.ascontiguousarray(xt8d.reshape(512, 2 * T))
        in_maps.append({
            "xtbf": np.ascontiguousarray(x[b][0:512].T).astype(bf),
            "xt8": xt8d.astype(f8),
            "wqbf": np.ascontiguousarray(wsel.T).astype(bf),
            "wq8": wq8d.astype(f8),
            "woutt": np.ascontiguousarray(w_out[:, gs].T).astype(bf),
            "trif8": pat.astype(f8),
            "tribf": pat.astype(bf),
            "negd8": negd.astype(f8),
            "negdbf": negd.astype(bf),
        })
    return in_maps


def kernel(x, mask, w_qkv, w_out, **_):
    nc = _get_nc()
    in_maps = make_in_maps(x, w_qkv, w_out)
    res = run_bass_kernel_spmd(nc, in_maps, core_ids=list(range(N_CORES)))
    y = np.zeros((B, T, DIM), dtype=np.float32)
    for c in range(N_CORES):
        y[c // 2] += res.results[c]["yt"].T
    return y
